# revision 1
# baseline (speedup 1.0000x reference)
"""Distributed Trainium2 Bass kernel for the associative-embedding (AE) loss.

Problem: per image b (B=8), two tag maps (tm0 [J,256,256], tm1 [J,512,512]),
keypoints kps [NH, 3*J] (x, y, vis interleaved, NH=30 humans, J=17 joints).
Per level: gather tag values at (j, x, y), masked per-human mean, pull loss
(masked squared deviation / num_humans) + push loss (pairwise Gaussian of
means / num_humans^2).  Output: per-image loss [B] (sum over both levels).

Strategy: pure data-parallel over B across 8 NeuronCores (core b handles
image b).  The loss touches only NH*J = 510 elements of each tag map, so
instead of streaming the 178 MB of tag maps, each core computes flat gather
indices on-chip from the keypoint data and pulls exactly 1020 scalars out
of DRAM via 8 indirect (SWDGE) DMAs of 128 single-element descriptors each
(HW indirect DMA = one descriptor per out partition row).  The gathered
values live in a [128, 8] chunk layout; one-hot matrices passed from the
host let the tensor engine reduce that layout directly into per-human
sufficient statistics (sum of masked vals, sum of masked vals^2), since
pull = sum(m v^2) - sv*avg.  Per-chunk stat products and matmuls are
pipelined under the remaining gathers.  The push loss uses a 32x32 DVE
stream transpose for the pairwise mean differences.  Per-core output is a
single scalar; the host stacks the 8 scalars into the final [8] vector.
"""

import numpy as np

B = 8
NH = 30
J = 17
H0 = W0 = 256
H1 = W1 = 512
N0 = J * H0 * W0
N1 = J * H1 * W1
NTOT = N0 + N1
NR = 2 * J * NH           # 1020 gathered elements
NC = 8                    # chunks of 128 (r = c*128 + p)
NI = 128 * NC
BIG = 1.0e9               # pad avg rows 30/31 -> exp(-BIG^2/2) = 0

_CACHE = {}

# ---------------------------------------------------------------------------
# host-side constants: chunk layout r = c*128 + p, f = r // NH, nh = r % NH
# ---------------------------------------------------------------------------


def _host_constants():
    if "consts" in _CACHE:
        return _CACHE["consts"]
    r = np.arange(NI)
    valid = r < NR
    f = np.where(valid, r // NH, 0)
    nh = np.where(valid, r % NH, 0)
    lvl = f // J
    j = f % J
    wmul = np.where(valid, np.where(lvl == 0, W0, W1), 0)
    base = np.where(valid, np.where(lvl == 0, j * H0 * W0, N0 + j * H1 * W1), 0)

    def chunkify(a):  # [NI] -> [128, NC]
        return np.ascontiguousarray(a.reshape(NC, 128).T)

    kcw = chunkify(wmul).astype(np.int32)
    kcb = chunkify(base).astype(np.int32)
    L0 = chunkify((valid & (lvl == 0)).astype(np.float32))
    L1 = chunkify((valid & (lvl == 1)).astype(np.float32))
    E = np.zeros((128, NC * NH), dtype=np.float32)
    for c in range(NC):
        rr = np.arange(c * 128, (c + 1) * 128)
        ok = rr < NR
        E[ok, c * NH + (rr[ok] % NH)] = 1.0
    cf = np.concatenate([L0, L1, E], axis=1).astype(np.float32)
    col_x = (lvl * 3 * J + 3 * j).astype(np.int64)
    _CACHE["consts"] = dict(
        kcw=kcw, kcb=kcb, cf=cf, nh=nh, col_x=col_x, valid=valid,
        chunkify=chunkify,
    )
    return _CACHE["consts"]


def make_in_maps(tag_maps0, tag_maps1, kps0, kps1):
    tag_maps0 = np.asarray(tag_maps0, dtype=np.float32)
    tag_maps1 = np.asarray(tag_maps1, dtype=np.float32)
    kps0 = np.asarray(kps0, dtype=np.int32)
    kps1 = np.asarray(kps1, dtype=np.int32)
    C = _host_constants()
    nh, col_x, valid = C["nh"], C["col_x"], C["valid"]
    chunkify = C["chunkify"]
    in_maps = []
    for b in range(B):
        tm = np.concatenate(
            [tag_maps0[b].ravel(), tag_maps1[b].ravel()]
        ).reshape(NTOT, 1)
        kp = np.concatenate([kps0[b], kps1[b]], axis=1)  # [30, 102]
        xs = np.zeros(NI, np.int32)
        ys = np.zeros(NI, np.int32)
        vs = np.zeros(NI, np.int32)
        xs[valid] = kp[nh[valid], col_x[valid]]
        ys[valid] = kp[nh[valid], col_x[valid] + 1]
        vs[valid] = kp[nh[valid], col_x[valid] + 2]
        kpg = np.stack(
            [chunkify(xs), chunkify(ys), chunkify(vs)], axis=2
        ).reshape(128, 3 * NC)
        ki = np.concatenate([kpg, C["kcw"], C["kcb"]], axis=1)  # [128, 40]
        k0 = np.ascontiguousarray(
            np.stack([kpg[:, 0], kpg[:, 1], C["kcw"][:, 0], C["kcb"][:, 0]],
                     axis=1)
        ).astype(np.int32)  # [128, 4]: x, y, W, base for chunk 0
        in_maps.append(
            {"tm": tm, "kp": kp, "ki": ki, "k0": k0, "cf": C["cf"]}
        )
    return in_maps


# ---------------------------------------------------------------------------
# device kernel (raw Block bass: hand-placed semaphores, no TileContext)
# ---------------------------------------------------------------------------


def _build_nc():
    from contextlib import ExitStack

    from concourse import bacc, mybir
    from concourse.bass import IndirectOffsetOnAxis

    f32 = mybir.dt.float32
    i32 = mybir.dt.int32
    Alu = mybir.AluOpType
    X = mybir.AxisListType.X
    Exp = mybir.ActivationFunctionType.Exp

    nc = bacc.Bacc()
    TM = nc.declare_dram_parameter("tm", [NTOT, 1], f32, isOutput=False)
    KP = nc.declare_dram_parameter("kp", [NH, 6 * J], i32, isOutput=False)
    KI = nc.declare_dram_parameter("ki", [128, 5 * NC], i32, isOutput=False)
    K0 = nc.declare_dram_parameter("k0", [128, 4], i32, isOutput=False)
    CF = nc.declare_dram_parameter(
        "cf", [128, 2 * NC + NC * NH], f32, isOutput=False
    )
    OUT = nc.declare_dram_parameter("out", [1, 1], f32, isOutput=True)

    with ExitStack() as ctx:
        e = ctx.enter_context
        kt = e(nc.sbuf_tensor("kt_sb", [NH, 6 * J], i32))
        ki = e(nc.sbuf_tensor("ki_sb", [128, 5 * NC], i32))
        k0 = e(nc.sbuf_tensor("k0_sb", [128, 4], i32))
        cf = e(nc.sbuf_tensor("cf_sb", [128, 2 * NC + NC * NH], f32))
        idxc = e(nc.sbuf_tensor("idxc", [128, NC], i32))
        xw = e(nc.sbuf_tensor("xw", [128, NC], i32))
        S = e(nc.sbuf_tensor("S", [128, 2 * NC], f32))
        maskg = e(nc.sbuf_tensor("maskg", [128, NC], f32))
        T = e(nc.sbuf_tensor("T", [128, 4 * NC], f32))
        sgq = e(nc.sbuf_tensor("sgq", [128, 2 * NC], f32))
        mgl = e(nc.sbuf_tensor("mgl", [128, 4 * NC], f32))
        maskf = e(nc.sbuf_tensor("maskf", [NH, 2 * J], f32))
        cnt = e(nc.sbuf_tensor("cnt", [NH, 2], f32))
        den = e(nc.sbuf_tensor("den", [NH, 2], f32))
        rden = e(nc.sbuf_tensor("rden", [NH, 2], f32))
        rdh = e(nc.sbuf_tensor("rdh", [NH, 2], f32))
        avg0 = e(nc.sbuf_tensor("avg0", [NH, 2], f32))
        u = e(nc.sbuf_tensor("u", [NH, 2], f32))
        avg32 = e(nc.sbuf_tensor("avg32", [32, 2], f32))
        avgsrc = e(nc.sbuf_tensor("avgsrc", [32, 64], f32))
        avgT = e(nc.sbuf_tensor("avgT", [32, 64], f32))
        d2 = e(nc.sbuf_tensor("d2", [NH, 64], f32))
        pm = e(nc.sbuf_tensor("pm", [NH, 64], f32))
        pack = e(nc.sbuf_tensor("pack", [NH, 6], f32))
        ones = e(nc.sbuf_tensor("ones", [NH, 1], f32))
        warm = e(nc.sbuf_tensor("warm", [1, 1], f32))
        rec = e(nc.sbuf_tensor("rec", [1, 6], f32))
        m1 = e(nc.sbuf_tensor("m1", [1, 4], f32))
        res = e(nc.sbuf_tensor("res", [1, 1], f32))
        ps_st = e(nc.psum_tensor("ps_st", [NH, 4], f32))
        ps_f = e(nc.psum_tensor("ps_f", [1, 6], f32))

        d_ki = e(nc.semaphore("d_ki"))
        d_k0 = e(nc.semaphore("d_k0"))
        d_kt = e(nc.semaphore("d_kt"))
        d_cf = e(nc.semaphore("d_cf"))
        d_out = e(nc.semaphore("d_out"))
        gs = [e(nc.semaphore(f"gs{c}")) for c in range(NC)]
        vdone = e(nc.semaphore("vdone"))
        a_exp = e(nc.semaphore("a_exp"))
        p_st = e(nc.semaphore("p_st"))
        p_f = e(nc.semaphore("p_f"))

        block = e(nc.Block())

        xg = ki[:, 0 : 3 * NC : 3]
        yg = ki[:, 1 : 3 * NC : 3]
        vg = ki[:, 2 : 3 * NC : 3]
        wmv = ki[:, 3 * NC : 4 * NC]
        bsv = ki[:, 4 * NC : 5 * NC]
        vis = kt[:, 2 : 6 * J : 3]

        # vdone markers filled in by the vector block, read by other blocks
        M = {}

        @block.vector
        def _(vector):
            n = 0

            def op(r, key=None):
                nonlocal n
                r.then_inc(vdone, 1)
                n += 1
                if key:
                    M[key] = n
                return n

            def wt(k):
                vector.wait_ge(vdone, k)

            op(vector.memset(warm[:], 0.0), "warm")
            op(vector.memset(avgsrc[:], BIG), "avgsrcinit")
            op(vector.memset(ones[:], 1.0), "ones")
            vector.wait_ge(d_k0, 16)
            # gather index, chunk 0 first (from the tiny k0 DMA).  xw = x*W
            # and idxc = y+base are independent; one waited join each.
            a = op(vector.tensor_tensor(
                out=idxc[:, 0:1], in0=k0[:, 1:2], in1=k0[:, 3:4], op=Alu.add))
            b = op(vector.tensor_tensor(
                out=xw[:, 0:1], in0=k0[:, 0:1], in1=k0[:, 2:3], op=Alu.mult))
            wt(b)
            op(vector.tensor_tensor(
                out=idxc[:, 0:1], in0=idxc[:, 0:1], in1=xw[:, 0:1],
                op=Alu.add), "idx0")
            vector.wait_ge(d_ki, 16)
            a = op(vector.tensor_tensor(
                out=idxc[:, 1:NC], in0=yg[:, 1:NC], in1=bsv[:, 1:NC],
                op=Alu.add))
            b = op(vector.tensor_tensor(
                out=xw[:, 1:NC], in0=xg[:, 1:NC], in1=wmv[:, 1:NC],
                op=Alu.mult))
            wt(b)
            op(vector.tensor_tensor(
                out=idxc[:, 1:NC], in0=idxc[:, 1:NC], in1=xw[:, 1:NC],
                op=Alu.add), "idx")
            op(vector.tensor_scalar(
                out=maskg[:], in0=vg, scalar1=0, scalar2=None, op0=Alu.is_gt),
                "maskg")
            # joint-count path (fills gather latency)
            vector.wait_ge(d_kt, 16)
            a = op(vector.tensor_scalar(
                out=maskf[:], in0=vis, scalar1=0, scalar2=None, op0=Alu.is_gt))
            wt(a)
            a = op(vector.reduce_sum(
                out=cnt[:], in_=maskf[:].rearrange("p (l j) -> p l j", l=2),
                axis=X))
            wt(a)
            a = op(vector.tensor_scalar(
                out=den[:], in0=cnt[:], scalar1=1.0, scalar2=None, op0=Alu.max))
            op(vector.tensor_scalar(
                out=pack[:, 4:6], in0=cnt[:], scalar1=0.0, scalar2=None,
                op0=Alu.is_gt), "has")
            wt(a)
            op(vector.reciprocal(rden[:], den[:]), "rden")
            wt(M["rden"])
            a = op(vector.tensor_tensor(
                out=rdh[:], in0=rden[:], in1=pack[:, 4:6], op=Alu.mult))
            wt(a)
            # premask the level indicators: mgl = maskg * [L0 | L1]
            vector.wait_ge(d_cf, 16)
            wt(M["maskg"])
            op(vector.tensor_tensor(
                out=mgl[:, 0:NC], in0=maskg[:], in1=cf[:, 0:NC], op=Alu.mult))
            a = op(vector.tensor_tensor(
                out=mgl[:, NC : 2 * NC], in0=maskg[:], in1=cf[:, NC : 2 * NC],
                op=Alu.mult))
            wt(a)
            a = op(vector.tensor_copy(
                out=mgl[:, 2 * NC : 4 * NC], in_=mgl[:, 0 : 2 * NC]))
            wt(a)
            for c in range(NC):
                vector.wait_ge(gs[c], 16)
                s2m = op(vector.tensor_tensor(
                    out=S[:, NC + c : NC + c + 1], in0=S[:, c : c + 1],
                    in1=S[:, c : c + 1], op=Alu.mult))
                wt(s2m)
                # T cols {c, 8+c, 16+c, 24+c} = [S, S, S2, S2]*[m0, m1, m0, m1]
                op(vector.tensor_tensor(
                    out=T[:, c : 4 * NC : NC].rearrange(
                        "p (q l) -> p q l", q=2),
                    in0=S[:, c : NC + c + 1 : NC].rearrange(
                        "p (q o) -> p q o", o=1).broadcast_to([128, 2, 2]),
                    in1=mgl[:, c : 4 * NC : NC].rearrange(
                        "p (q l) -> p q l", q=2),
                    op=Alu.mult), f"T{c}")
            # averages: avg32 first -- it gates the long push chain.
            # rdenh = rden*has precomputed off-path would help, but rden and
            # has are both ready; fold them here via avg0 ordering instead.
            vector.wait_ge(p_st, 1)
            sv = ps_st[:, 0:2]
            s2 = ps_st[:, 2:4]
            avm = op(vector.tensor_tensor(
                out=avgsrc[0:NH, :].rearrange("p (l j) -> p l j", l=2),
                in0=sv.to_broadcast([NH, 2, 32]),
                in1=rdh[:].to_broadcast([NH, 2, 32]),
                op=Alu.mult))
            wt(avm)
            a = op(vector.transpose(avgT[:], avgsrc[:]))
            wt(a)
            a = op(vector.tensor_tensor(
                out=d2[:], in0=avgT[0:NH, :], in1=avgsrc[0:NH, :],
                op=Alu.subtract))
            wt(a)
            op(vector.tensor_tensor(
                out=d2[:], in0=d2[:], in1=d2[:], op=Alu.mult), "d2")
            # pull (overlaps the ACT exp): pull = s2 - sv*avg
            um = op(vector.tensor_tensor(
                out=u[:], in0=sv,
                in1=avgsrc[0:NH, :].rearrange(
                    "p (l j) -> p l j", l=2)[:, :, 0:1].rearrange(
                    "p l o -> p (l o)"),
                op=Alu.mult))
            wt(um)
            op(vector.tensor_tensor(
                out=pack[:, 0:4:2], in0=s2, in1=u[:], op=Alu.subtract),
                "pull")
            # push row sums once ACT finished the exp
            vector.wait_ge(a_exp, 1)
            op(vector.reduce_sum(
                out=pack[:, 1:4:2],
                in_=pm[:].rearrange("p (l j) -> p l j", l=2), axis=X),
                "push")
            # final scalar
            vector.wait_ge(p_f, 1)
            a = op(vector.reciprocal(rec[:], ps_f[:]))
            wt(a)
            a = op(vector.tensor_tensor(
                out=m1[:].rearrange("p (l q) -> p l q", l=2),
                in0=ps_f[:, 0:4].rearrange("p (l q) -> p l q", l=2),
                in1=rec[:, 4:6].to_broadcast([1, 2, 2]), op=Alu.mult))
            wt(a)
            a = op(vector.tensor_tensor(
                out=m1[:, 1:4:2], in0=m1[:, 1:4:2], in1=rec[:, 4:6],
                op=Alu.mult))
            wt(a)
            op(vector.reduce_sum(out=res[:], in_=m1[:], axis=X), "res")

        @block.sync
        def _(sync):
            sync.dma_start(out=k0[:], in_=K0[:]).then_inc(d_k0, 16)
            sync.dma_start(out=ki[:], in_=KI[:]).then_inc(d_ki, 16)
            sync.dma_start(out=cf[:], in_=CF[:]).then_inc(d_cf, 16)
            sync.dma_start(out=kt[:], in_=KP[:]).then_inc(d_kt, 16)
            sync.wait_ge(vdone, M["res"])
            sync.dma_start(out=OUT[:], in_=res[:]).then_inc(d_out, 16)
            sync.wait_ge(d_out, 16)

        @block.gpsimd
        def _(gpsimd):
            gpsimd.wait_ge(vdone, M["idx0"])
            gpsimd.indirect_dma_start(
                out=S[:, 0:1],
                out_offset=None,
                in_=TM[:],
                in_offset=IndirectOffsetOnAxis(ap=idxc[:, 0:1], axis=0),
            ).then_inc(gs[0], 16)
            gpsimd.wait_ge(vdone, M["idx"])
            for c in range(1, NC):
                gpsimd.indirect_dma_start(
                    out=S[:, c : c + 1],
                    out_offset=None,
                    in_=TM[:],
                    in_offset=IndirectOffsetOnAxis(
                        ap=idxc[:, c : c + 1], axis=0
                    ),
                ).then_inc(gs[c], 16)

        @block.scalar
        def _(scalar):
            scalar.wait_ge(vdone, M["warm"])
            scalar.activation(warm[:], warm[:], Exp)
            scalar.wait_ge(vdone, M["d2"])
            scalar.activation(
                pm[:], d2[:], Exp, scale=-0.5
            ).then_inc(a_exp, 1)

        @block.tensor
        def _(tensor):
            tensor.wait_ge(d_cf, 16)
            for c in range(NC):
                tensor.wait_ge(vdone, M[f"T{c}"])
                mm = tensor.matmul(
                    ps_st[:],
                    lhsT=cf[:, 2 * NC + c * NH : 2 * NC + (c + 1) * NH],
                    rhs=T[:, c : 4 * NC : NC],
                    start=(c == 0),
                    stop=(c == NC - 1),
                )
            mm.then_inc(p_st, 1)
            tensor.wait_ge(vdone, M["push"])
            tensor.matmul(
                ps_f[:], lhsT=ones[:], rhs=pack[:], start=True, stop=True
            ).then_inc(p_f, 1)

    nc.finalize()
    return nc


def _get_nc():
    if "nc" not in _CACHE:
        _CACHE["nc"] = _build_nc()
    return _CACHE["nc"]


def kernel(tag_maps0, tag_maps1, kps0, kps1):
    from concourse.bass_utils import run_bass_kernel_spmd

    nc = _get_nc()
    in_maps = make_in_maps(tag_maps0, tag_maps1, kps0, kps1)
    out = run_bass_kernel_spmd(nc, in_maps, core_ids=list(range(B)))
    return np.array(
        [np.asarray(out.results[b]["out"]).reshape(()) for b in range(B)],
        dtype=np.float32,
    )



# revision 8
# speedup vs baseline: 1.1362x; 1.1362x over previous
"""Distributed Trainium2 Bass kernel for the associative-embedding (AE) loss.

Problem: per image b (B=8), two tag maps (tm0 [J,256,256], tm1 [J,512,512]),
keypoints kps [NH, 3*J] (x, y, vis interleaved, NH=30 humans, J=17 joints).
Per level: gather tag values at (j, x, y), masked per-human mean, pull loss
(masked squared deviation / num_humans) + push loss (pairwise Gaussian of
means / num_humans^2).  Output: per-image loss [B] (sum over both levels).

Strategy: pure data-parallel over B across 8 NeuronCores (core b handles
image b).  The loss touches only the ~NH*2*J visible-keypoint elements of
the 22 MB of tag maps, so each core pulls exactly those scalars out of DRAM
via indirect (SWDGE) DMAs.  The HW indirect DMA emits one descriptor per
out-partition row (max 128 scattered elements per ~1.1 us instruction), so
the host packs ONLY the visible entries into ceil(V/128) chunks of 128 --
typically 5 instead of 8 for the full grid -- and bakes one-hot human/level
matrices so the tensor engine reduces the chunk layout into per-human
sufficient statistics (sum, sum of squares) while later gathers are still
in flight.  gpsimd fetches its own index table via SWDGE (faster first-DMA
path than HWDGE), sync fetches the fp32 constants concurrently.  The push
loss uses a 32x32 DVE stream transpose, Square-with-bias + one Exp on the
scalar engine, a ones-vector matmul, and a 2-op weighted reduce; all
input-only quantities (masks, reciprocal counts, 1/num_humans weights) are
host-precomputed.  Per-core output is one scalar; the host stacks the 8
scalars into the final [B] vector.
"""

import numpy as np

B = 8
NH = 30
J = 17
H0 = W0 = 256
H1 = W1 = 512
N0 = J * H0 * W0
N1 = J * H1 * W1
NTOT = N0 + N1
BIG = 30.0                # pad rows -> exp(-(BIG+avg)^2/2) ~ 1e-170 ~ 0
                          # (kept small: ACT Square is a piecewise table and
                          # must stay accurate at BIG, unlike huge sentinels)

_CACHE = {}


# ---------------------------------------------------------------------------
# host-side input prep: valid-packed gather indices + one-hot reduction maps
# ---------------------------------------------------------------------------


def make_in_maps(tag_maps0, tag_maps1, kps0, kps1):
    tag_maps0 = np.asarray(tag_maps0, dtype=np.float32)
    tag_maps1 = np.asarray(tag_maps1, dtype=np.float32)
    kps0 = np.asarray(kps0, dtype=np.int64)
    kps1 = np.asarray(kps1, dtype=np.int64)
    jr = np.arange(J)[None, :]
    per_img = []
    nv_max = 0
    for b in range(B):
        xs0, ys0, vs0 = kps0[b, :, 0::3], kps0[b, :, 1::3], kps0[b, :, 2::3]
        xs1, ys1, vs1 = kps1[b, :, 0::3], kps1[b, :, 1::3], kps1[b, :, 2::3]
        idx_hlj = np.concatenate(
            [jr * (H0 * W0) + xs0 * W0 + ys0,
             N0 + jr * (H1 * W1) + xs1 * W1 + ys1], axis=1
        )  # [30, 34] flat index per (human, level*J+joint)
        mask = np.concatenate([vs0 != 0, vs1 != 0], axis=1)  # [30, 34] bool
        hh, cc = np.nonzero(mask)     # valid entries: human, level*J+joint
        per_img.append((idx_hlj, mask, hh, cc))
        nv_max = max(nv_max, len(hh))
    NC = -(-nv_max // 128)            # chunks of 128 descriptors

    in_maps = []
    for b in range(B):
        idx_hlj, mask, hh, cc = per_img[b]
        nv = len(hh)
        idxc = np.zeros((128, NC), np.int32)
        E = np.zeros((128, NC * NH), np.float32)
        L4 = np.zeros((128, 4 * NC), np.float32)  # per chunk: (q, l) cols
        r = np.arange(nv)
        ch, p = r // 128, r % 128
        lvl = (cc // J).astype(np.int64)
        idxc[p, ch] = idx_hlj[hh, cc]
        E[p, ch * NH + hh] = 1.0
        L4[p, ch * 4 + 0 + lvl] = 1.0          # q=0 (S * L_l)
        L4[p, ch * 4 + 2 + lvl] = 1.0          # q=1 (S^2 * L_l)

        cnt = np.stack([mask[:, :J].sum(1), mask[:, J:].sum(1)], 1).astype(
            np.float32
        )
        has = (cnt > 0).astype(np.float32)
        rdh = has / np.maximum(cnt, 1.0)
        P = 1.0 / has.sum(0)          # [2] 1/num_humans per level
        # kf layout [128, 4*NC + NC*NH + 8]:
        #   cols 0:4NC           L4
        #   cols 4NC:4NC+NC*NH   E
        #   next 2: -rdh | next 2: rdh | next 4 (row 0): w4
        kf = np.zeros((128, 4 * NC + NC * NH + 8), np.float32)
        kf[:, 0 : 4 * NC] = L4
        kf[:, 4 * NC : 4 * NC + NC * NH] = E
        base = 4 * NC + NC * NH
        kf[0:NH, base : base + 2] = -rdh
        kf[0:NH, base + 2 : base + 4] = rdh
        kf[0, base + 4 : base + 8] = [P[0] ** 2, P[1] ** 2, P[0], P[1]]
        tm = np.concatenate(
            [tag_maps0[b].ravel(), tag_maps1[b].ravel()]
        ).reshape(NTOT, 1)
        in_maps.append({"tm": tm, "ki": idxc, "kf": kf})
    return in_maps, NC


# ---------------------------------------------------------------------------
# device kernel (raw Block bass: hand-placed semaphores, no TileContext)
# ---------------------------------------------------------------------------


def _build_nc(NC):
    from contextlib import ExitStack

    from concourse import bacc, mybir
    from concourse.bass import IndirectOffsetOnAxis

    f32 = mybir.dt.float32
    i32 = mybir.dt.int32
    Alu = mybir.AluOpType
    X = mybir.AxisListType.X
    Exp = mybir.ActivationFunctionType.Exp
    Square = mybir.ActivationFunctionType.Square
    KFW = 4 * NC + NC * NH + 8
    base = 4 * NC + NC * NH

    nc = bacc.Bacc()
    TM = nc.declare_dram_parameter("tm", [NTOT, 1], f32, isOutput=False)
    KI = nc.declare_dram_parameter("ki", [128, NC], i32, isOutput=False)
    KF = nc.declare_dram_parameter("kf", [128, KFW], f32, isOutput=False)
    OUT = nc.declare_dram_parameter("out", [1, 1], f32, isOutput=True)

    with ExitStack() as ctx:
        e = ctx.enter_context
        ki = e(nc.sbuf_tensor("ki_sb", [128, NC], i32))
        kf = e(nc.sbuf_tensor("kf_sb", [128, KFW], f32))
        SS = e(nc.sbuf_tensor("SS", [128, 2 * NC], f32))  # [S | S^2]
        T = e(nc.sbuf_tensor("T", [128, 4 * NC], f32))
        avg = e(nc.sbuf_tensor("avg", [NH, 2], f32))
        u = e(nc.sbuf_tensor("u", [NH, 2], f32))
        avgsrc = e(nc.sbuf_tensor("avgsrc", [32, 64], f32))
        avgT = e(nc.sbuf_tensor("avgT", [32, 64], f32))
        d2 = e(nc.sbuf_tensor("d2", [NH, 64], f32))
        pm = e(nc.sbuf_tensor("pm", [NH, 64], f32))
        Z = e(nc.sbuf_tensor("Z", [NH, 4], f32))
        ones = e(nc.sbuf_tensor("ones", [NH, 1], f32))
        warm = e(nc.sbuf_tensor("warm", [1, 2], f32))
        fin = e(nc.sbuf_tensor("fin", [1, 4], f32))
        res = e(nc.sbuf_tensor("res", [1, 1], f32))
        ps_st = e(nc.psum_tensor("ps_st", [NH, 4], f32))
        ps_f = e(nc.psum_tensor("ps_f", [1, 4], f32))

        d_ki = e(nc.semaphore("d_ki"))
        d_kf = e(nc.semaphore("d_kf"))
        gs = [e(nc.semaphore(f"gs{c}")) for c in range(NC)]
        vdone = e(nc.semaphore("vdone"))
        adone = e(nc.semaphore("adone"))
        a_exp = e(nc.semaphore("a_exp"))
        p_st = e(nc.semaphore("p_st"))
        p_f = e(nc.semaphore("p_f"))
        d_out = e(nc.semaphore("d_out"))

        block = e(nc.Block())
        M = {}

        @block.gpsimd
        def _(gpsimd):
            # SWDGE fetch of the index table: faster first-DMA path than
            # HWDGE and keeps the whole gather chain on one engine.
            gpsimd.dma_start(out=ki[:], in_=KI[:]).then_inc(d_ki, 16)
            gpsimd.wait_ge(d_ki, 16)
            for c in range(NC):
                gpsimd.indirect_dma_start(
                    out=SS[:, c : c + 1],
                    out_offset=None,
                    in_=TM[:],
                    in_offset=IndirectOffsetOnAxis(
                        ap=ki[:, c : c + 1], axis=0
                    ),
                ).then_inc(gs[c], 16)

        @block.vector
        def _(vector):
            n = 0

            def op(r, key=None):
                nonlocal n
                r.then_inc(vdone, 1)
                n += 1
                if key:
                    M[key] = n
                return n

            def wt(k):
                vector.wait_ge(vdone, k)

            op(vector.memset(avgsrc[:], BIG))
            op(vector.memset(ones[:], 1.0))
            op(vector.memset(warm[:], 0.0), "warm")
            vector.wait_ge(d_kf, 16)
            # per chunk: S^2 then T[:, 4c:4c+4] = [S,S,S2,S2]*[L0,L1,L0,L1]
            for c in range(NC):
                vector.wait_ge(gs[c], 16)
                a = op(vector.tensor_tensor(
                    out=SS[:, NC + c : NC + c + 1],
                    in0=SS[:, c : c + 1], in1=SS[:, c : c + 1], op=Alu.mult))
                wt(a)
                op(vector.tensor_tensor(
                    out=T[:, 4 * c : 4 * c + 4].rearrange(
                        "p (q l) -> p q l", q=2),
                    in0=SS[:, c : NC + c + 1 : NC].rearrange(
                        "p (q o) -> p q o", o=1).broadcast_to([128, 2, 2]),
                    in1=kf[:, 4 * c : 4 * c + 4].rearrange(
                        "p (q l) -> p q l", q=2),
                    op=Alu.mult), f"T{c}")
            # stats landed in psum: avg chain drives the push tail
            vector.wait_ge(p_st, 1)
            avg_n = op(vector.tensor_tensor(
                out=avg[:], in0=ps_st[:, 0:2],
                in1=kf[0:NH, base + 2 : base + 4], op=Alu.mult))
            a = op(vector.tensor_tensor(
                out=avgsrc[0:NH, :].rearrange("p (l j) -> p l j", l=2),
                in0=ps_st[:, 0:2].to_broadcast([NH, 2, 32]),
                in1=kf[0:NH, base : base + 2].to_broadcast([NH, 2, 32]),
                op=Alu.mult))
            wt(a)
            op(vector.transpose(avgT[:], avgsrc[:]), "tr")
            # pull stats while ACT runs the push exp
            wt(avg_n)
            u_n = op(vector.tensor_tensor(
                out=u[:], in0=ps_st[:, 0:2], in1=avg[:], op=Alu.mult))
            wt(u_n)
            op(vector.tensor_tensor(
                out=Z[:, 2:4], in0=ps_st[:, 2:4], in1=u[:],
                op=Alu.subtract), "pull")
            # push row block sums once ACT finished the exp
            vector.wait_ge(a_exp, 1)
            op(vector.reduce_sum(
                out=Z[:, 0:2],
                in_=pm[:].rearrange("p (l j) -> p l j", l=2), axis=X),
                "push")
            # final weighted reduce of [push0, push1, pull0, pull1]
            vector.wait_ge(p_f, 1)
            a = op(vector.tensor_tensor(
                out=fin[:], in0=ps_f[:], in1=kf[0:1, base + 4 : base + 8],
                op=Alu.mult))
            wt(a)
            op(vector.reduce_sum(out=res[:], in_=fin[:], axis=X), "res")

        @block.sync
        def _(sync):
            sync.dma_start(out=kf[:], in_=KF[:]).then_inc(d_kf, 16)
            sync.wait_ge(vdone, M["res"])
            sync.dma_start(out=OUT[:], in_=res[:]).then_inc(d_out, 16)
            sync.wait_ge(d_out, 16)

        @block.scalar
        def _(scalar):
            scalar.wait_ge(vdone, M["warm"])
            scalar.activation(warm[:, 0:1], warm[:, 0:1], Exp).then_inc(
                adone, 1
            )
            scalar.activation(warm[:, 1:2], warm[:, 1:2], Square).then_inc(
                adone, 1
            )
            scalar.wait_ge(vdone, M["tr"])
            # d2[i, l*32+j] = (avg_j - avg_i)^2 ; avgT holds -avg_j, bias +avg_i
            scalar.activation(
                d2[:, 0:32], avgT[0:NH, 0:32], Square, bias=avg[:, 0:1]
            ).then_inc(adone, 1)
            scalar.activation(
                d2[:, 32:64], avgT[0:NH, 32:64], Square, bias=avg[:, 1:2]
            ).then_inc(adone, 1)
            scalar.wait_ge(adone, 4)
            scalar.activation(
                pm[:], d2[:], Exp, scale=-0.5
            ).then_inc(a_exp, 1)

        @block.tensor
        def _(tensor):
            for c in range(NC):
                tensor.wait_ge(vdone, M[f"T{c}"])
                mm = tensor.matmul(
                    ps_st[:],
                    lhsT=kf[:, 4 * NC + c * NH : 4 * NC + (c + 1) * NH],
                    rhs=T[:, 4 * c : 4 * c + 4],
                    start=(c == 0),
                    stop=(c == NC - 1),
                )
            mm.then_inc(p_st, 1)
            tensor.wait_ge(vdone, M["push"])
            tensor.matmul(
                ps_f[:], lhsT=ones[:], rhs=Z[:], start=True, stop=True
            ).then_inc(p_f, 1)

    nc.finalize()
    return nc


def _get_nc(NC):
    if NC not in _CACHE:
        _CACHE[NC] = _build_nc(NC)
    return _CACHE[NC]


def kernel(tag_maps0, tag_maps1, kps0, kps1):
    from concourse.bass_utils import run_bass_kernel_spmd

    in_maps, NC = make_in_maps(tag_maps0, tag_maps1, kps0, kps1)
    nc = _get_nc(NC)
    out = run_bass_kernel_spmd(nc, in_maps, core_ids=list(range(B)))
    return np.array(
        [np.asarray(out.results[b]["out"]).reshape(()) for b in range(B)],
        dtype=np.float32,
    )


# revision 9
# speedup vs baseline: 1.2229x; 1.0763x over previous
"""Distributed Trainium2 Bass kernel for the associative-embedding (AE) loss.

Problem: per image b (B=8), two tag maps (tm0 [J,256,256], tm1 [J,512,512]),
keypoints kps [NH, 3*J] (x, y, vis interleaved, NH=30 humans, J=17 joints).
Per level: gather tag values at (j, x, y), masked per-human mean, pull loss
(masked squared deviation / num_humans) + push loss (pairwise Gaussian of
means / num_humans^2).  Output: per-image loss [B] (sum over both levels).

Strategy: pure data-parallel over B across 8 NeuronCores (core b handles
image b).  The loss touches only the ~NH*2*J visible-keypoint elements of
the 22 MB of tag maps, so each core pulls exactly those scalars out of DRAM
via indirect (SWDGE) DMAs.  The HW indirect DMA emits one descriptor per
out-partition row (max 128 scattered elements per ~1.1 us instruction), so
the host packs ONLY the visible entries into ceil(V/128) chunks -- typically
5 instead of 8 for the full grid -- with the small remainder chunk LAST so
its data drains quickly after the final descriptor-generation burst.  Host-
baked one-hot matrices let the tensor engine reduce the chunk layout into
per-human sufficient statistics via lhsT = E*S, rhs = [L0, L1, S*L0, S*L1]
(the two DVE prep ops per chunk are mutually independent, so the last
chunk's critical path is one op deep); chunks are processed while later
gathers are still in flight.  The push loss uses a 32x32 DVE stream
transpose, Square-with-bias + one Exp on the scalar engine, a ones-vector
matmul, and a 2-op weighted reduce; all input-only quantities (masks,
reciprocal counts, 1/num_humans weights) are host-precomputed.  Per-core
output is one scalar; the host stacks the 8 scalars into the final [B]
vector.
"""

import numpy as np

B = 8
NH = 30
J = 17
H0 = W0 = 256
H1 = W1 = 512
N0 = J * H0 * W0
N1 = J * H1 * W1
NTOT = N0 + N1
BIG = 30.0                # pad rows -> exp(-(BIG+avg)^2/2) ~ 1e-170 ~ 0
                          # (kept small: ACT Square is a piecewise table and
                          # must stay accurate at BIG, unlike huge sentinels)

_CACHE = {}


# ---------------------------------------------------------------------------
# host-side input prep: valid-packed gather indices + one-hot reduction maps
# ---------------------------------------------------------------------------


def make_in_maps(tag_maps0, tag_maps1, kps0, kps1):
    tag_maps0 = np.asarray(tag_maps0, dtype=np.float32)
    tag_maps1 = np.asarray(tag_maps1, dtype=np.float32)
    kps0 = np.asarray(kps0, dtype=np.int64)
    kps1 = np.asarray(kps1, dtype=np.int64)
    jr = np.arange(J)[None, :]
    per_img = []
    nv_max = 0
    for b in range(B):
        xs0, ys0, vs0 = kps0[b, :, 0::3], kps0[b, :, 1::3], kps0[b, :, 2::3]
        xs1, ys1, vs1 = kps1[b, :, 0::3], kps1[b, :, 1::3], kps1[b, :, 2::3]
        idx_hlj = np.concatenate(
            [jr * (H0 * W0) + xs0 * W0 + ys0,
             N0 + jr * (H1 * W1) + xs1 * W1 + ys1], axis=1
        )  # [30, 34] flat index per (human, level*J+joint)
        mask = np.concatenate([vs0 != 0, vs1 != 0], axis=1)  # [30, 34] bool
        hh, cc = np.nonzero(mask)     # valid entries: human, level*J+joint
        per_img.append((idx_hlj, mask, hh, cc))
        nv_max = max(nv_max, len(hh))
    NC = -(-nv_max // 128)            # chunks of <=128 descriptors
    n_last = nv_max - 128 * (NC - 1)  # last (remainder) chunk size

    in_maps = []
    for b in range(B):
        idx_hlj, mask, hh, cc = per_img[b]
        nv = len(hh)
        idxc = np.zeros((128, NC), np.int32)
        E = np.zeros((128, NC * NH), np.float32)
        T = np.zeros((128, 4 * NC), np.float32)  # [L0, L1, 0, 0] per chunk
        r = np.arange(nv)
        ch, p = r // 128, r % 128
        lvl = (cc // J).astype(np.int64)
        idxc[p, ch] = idx_hlj[hh, cc]
        E[p, ch * NH + hh] = 1.0
        T[p, ch * 4 + lvl] = 1.0

        cnt = np.stack([mask[:, :J].sum(1), mask[:, J:].sum(1)], 1).astype(
            np.float32
        )
        has = (cnt > 0).astype(np.float32)
        rdh = has / np.maximum(cnt, 1.0)
        P = 1.0 / has.sum(0)          # [2] 1/num_humans per level
        # kf layout [128, 4*NC + NC*NH + 8]:
        #   cols 0:4NC            T/rhs region (host: L0,L1,0,0; DVE: S*L)
        #   cols 4NC:4NC+NC*NH    E one-hot
        #   next 2: -rdh | next 2: rdh | next 4 (row 0): w4
        kf = np.zeros((128, 4 * NC + NC * NH + 8), np.float32)
        kf[:, 0 : 4 * NC] = T
        kf[:, 4 * NC : 4 * NC + NC * NH] = E
        base = 4 * NC + NC * NH
        kf[0:NH, base : base + 2] = -rdh
        kf[0:NH, base + 2 : base + 4] = rdh
        kf[0, base + 4 : base + 8] = [P[0] ** 2, P[1] ** 2, P[0], P[1]]
        tm = np.concatenate(
            [tag_maps0[b].ravel(), tag_maps1[b].ravel()]
        ).reshape(NTOT, 1)
        in_maps.append({"tm": tm, "ki": idxc, "kf": kf})
    return in_maps, NC, n_last


# ---------------------------------------------------------------------------
# device kernel (raw Block bass: hand-placed semaphores, no TileContext)
# ---------------------------------------------------------------------------


def _build_nc(NC, n_last):
    from contextlib import ExitStack

    from concourse import bacc, mybir
    from concourse.bass import IndirectOffsetOnAxis

    f32 = mybir.dt.float32
    i32 = mybir.dt.int32
    Alu = mybir.AluOpType
    X = mybir.AxisListType.X
    Exp = mybir.ActivationFunctionType.Exp
    Square = mybir.ActivationFunctionType.Square
    KFW = 4 * NC + NC * NH + 8
    base = 4 * NC + NC * NH

    nc = bacc.Bacc()
    TM = nc.declare_dram_parameter("tm", [NTOT, 1], f32, isOutput=False)
    KI = nc.declare_dram_parameter("ki", [128, NC], i32, isOutput=False)
    KF = nc.declare_dram_parameter("kf", [128, KFW], f32, isOutput=False)
    OUT = nc.declare_dram_parameter("out", [1, 1], f32, isOutput=True)

    with ExitStack() as ctx:
        e = ctx.enter_context
        ki = e(nc.sbuf_tensor("ki_sb", [128, NC], i32))
        kf = e(nc.sbuf_tensor("kf_sb", [128, KFW], f32))
        S = e(nc.sbuf_tensor("S", [128, NC], f32))
        EST = e(nc.sbuf_tensor("EST", [128, NC * NH], f32))
        avg = e(nc.sbuf_tensor("avg", [NH, 2], f32))
        u = e(nc.sbuf_tensor("u", [NH, 2], f32))
        avgsrc = e(nc.sbuf_tensor("avgsrc", [32, 64], f32))
        avgT = e(nc.sbuf_tensor("avgT", [32, 64], f32))
        d2 = e(nc.sbuf_tensor("d2", [NH, 64], f32))
        pm = e(nc.sbuf_tensor("pm", [NH, 64], f32))
        Z = e(nc.sbuf_tensor("Z", [NH, 4], f32))
        ones = e(nc.sbuf_tensor("ones", [NH, 1], f32))
        warm = e(nc.sbuf_tensor("warm", [1, 2], f32))
        fin = e(nc.sbuf_tensor("fin", [1, 4], f32))
        res = e(nc.sbuf_tensor("res", [1, 1], f32))
        ps_st = e(nc.psum_tensor("ps_st", [NH, 4], f32))
        ps_f = e(nc.psum_tensor("ps_f", [1, 4], f32))

        d_ki = e(nc.semaphore("d_ki"))
        d_kf = e(nc.semaphore("d_kf"))
        gs = [e(nc.semaphore(f"gs{c}")) for c in range(NC)]
        vdone = e(nc.semaphore("vdone"))
        adone = e(nc.semaphore("adone"))
        a_exp = e(nc.semaphore("a_exp"))
        p_st = e(nc.semaphore("p_st"))
        p_f = e(nc.semaphore("p_f"))
        d_out = e(nc.semaphore("d_out"))

        block = e(nc.Block())
        M = {}

        @block.vector
        def _(vector):
            n = 0

            def op(r, key=None):
                nonlocal n
                r.then_inc(vdone, 1)
                n += 1
                if key:
                    M[key] = n
                return n

            def wt(k):
                vector.wait_ge(vdone, k)

            # S zeroed so the partial last chunk's unwritten rows can't
            # inject NaNs through the 0-masked products
            op(vector.memset(S[:], 0.0), "sz")
            op(vector.memset(avgsrc[:], BIG))
            op(vector.memset(ones[:], 1.0))
            op(vector.memset(warm[:], 0.0), "warm")
            vector.wait_ge(d_kf, 16)
            # per chunk: rhs cols 2:4 = S*[L0,L1]; lhsT = E*S (independent)
            for c in range(NC):
                vector.wait_ge(gs[c], 16)
                op(vector.tensor_tensor(
                    out=kf[:, 4 * c + 2 : 4 * c + 4].rearrange(
                        "p (o l) -> p o l", o=1),
                    in0=S[:, c : c + 1].to_broadcast([128, 1, 2]),
                    in1=kf[:, 4 * c : 4 * c + 2].rearrange(
                        "p (o l) -> p o l", o=1),
                    op=Alu.mult))
                op(vector.tensor_tensor(
                    out=EST[:, c * NH : (c + 1) * NH].rearrange(
                        "p (o h) -> p o h", o=1),
                    in0=S[:, c : c + 1].to_broadcast([128, 1, NH]),
                    in1=kf[:, 4 * NC + c * NH : 4 * NC + (c + 1) * NH
                           ].rearrange("p (o h) -> p o h", o=1),
                    op=Alu.mult), f"T{c}")
            # stats landed in psum: avg chain drives the push tail
            vector.wait_ge(p_st, 1)
            avg_n = op(vector.tensor_tensor(
                out=avg[:], in0=ps_st[:, 0:2],
                in1=kf[0:NH, base + 2 : base + 4], op=Alu.mult))
            a = op(vector.tensor_tensor(
                out=avgsrc[0:NH, :].rearrange("p (l j) -> p l j", l=2),
                in0=ps_st[:, 0:2].to_broadcast([NH, 2, 32]),
                in1=kf[0:NH, base : base + 2].to_broadcast([NH, 2, 32]),
                op=Alu.mult))
            wt(a)
            op(vector.transpose(avgT[:], avgsrc[:]), "tr")
            # pull stats while ACT runs the push exp
            wt(avg_n)
            u_n = op(vector.tensor_tensor(
                out=u[:], in0=ps_st[:, 0:2], in1=avg[:], op=Alu.mult))
            wt(u_n)
            op(vector.tensor_tensor(
                out=Z[:, 2:4], in0=ps_st[:, 2:4], in1=u[:],
                op=Alu.subtract), "pull")
            # push row block sums once ACT finished the exp
            vector.wait_ge(a_exp, 1)
            op(vector.reduce_sum(
                out=Z[:, 0:2],
                in_=pm[:].rearrange("p (l j) -> p l j", l=2), axis=X),
                "push")
            # final weighted reduce of [push0, push1, pull0, pull1]
            vector.wait_ge(p_f, 1)
            a = op(vector.tensor_tensor(
                out=fin[:], in0=ps_f[:], in1=kf[0:1, base + 4 : base + 8],
                op=Alu.mult))
            wt(a)
            op(vector.reduce_sum(out=res[:], in_=fin[:], axis=X), "res")

        @block.sync
        def _(sync):
            sync.dma_start(out=ki[:], in_=KI[:]).then_inc(d_ki, 16)
            sync.dma_start(out=kf[:], in_=KF[:]).then_inc(d_kf, 16)
            sync.wait_ge(vdone, M["res"])
            sync.dma_start(out=OUT[:], in_=res[:]).then_inc(d_out, 16)
            sync.wait_ge(d_out, 16)

        @block.gpsimd
        def _(gpsimd):
            gpsimd.wait_ge(d_ki, 16)
            gpsimd.wait_ge(vdone, M["sz"])
            for c in range(NC):
                rows = 128 if c < NC - 1 else n_last
                gpsimd.indirect_dma_start(
                    out=S[0:rows, c : c + 1],
                    out_offset=None,
                    in_=TM[:],
                    in_offset=IndirectOffsetOnAxis(
                        ap=ki[0:rows, c : c + 1], axis=0
                    ),
                ).then_inc(gs[c], 16)

        @block.scalar
        def _(scalar):
            scalar.wait_ge(vdone, M["warm"])
            scalar.activation(warm[:, 0:1], warm[:, 0:1], Exp).then_inc(
                adone, 1
            )
            scalar.activation(warm[:, 1:2], warm[:, 1:2], Square).then_inc(
                adone, 1
            )
            scalar.wait_ge(vdone, M["tr"])
            # d2[i, l*32+j] = (avg_j - avg_i)^2 ; avgT holds -avg_j, bias +avg_i
            scalar.activation(
                d2[:, 0:32], avgT[0:NH, 0:32], Square, bias=avg[:, 0:1]
            ).then_inc(adone, 1)
            scalar.activation(
                d2[:, 32:64], avgT[0:NH, 32:64], Square, bias=avg[:, 1:2]
            ).then_inc(adone, 1)
            scalar.wait_ge(adone, 4)
            scalar.activation(
                pm[:], d2[:], Exp, scale=-0.5
            ).then_inc(a_exp, 1)

        @block.tensor
        def _(tensor):
            for c in range(NC):
                tensor.wait_ge(vdone, M[f"T{c}"])
                mm = tensor.matmul(
                    ps_st[:],
                    lhsT=EST[:, c * NH : (c + 1) * NH],
                    rhs=kf[:, 4 * c : 4 * c + 4],
                    start=(c == 0),
                    stop=(c == NC - 1),
                )
            mm.then_inc(p_st, 1)
            tensor.wait_ge(vdone, M["push"])
            tensor.matmul(
                ps_f[:], lhsT=ones[:], rhs=Z[:], start=True, stop=True
            ).then_inc(p_f, 1)

    nc.finalize()
    return nc


def _get_nc(NC, n_last):
    key = (NC, n_last)
    if key not in _CACHE:
        _CACHE[key] = _build_nc(NC, n_last)
    return _CACHE[key]


def kernel(tag_maps0, tag_maps1, kps0, kps1):
    from concourse.bass_utils import run_bass_kernel_spmd

    in_maps, NC, n_last = make_in_maps(tag_maps0, tag_maps1, kps0, kps1)
    nc = _get_nc(NC, n_last)
    out = run_bass_kernel_spmd(nc, in_maps, core_ids=list(range(B)))
    return np.array(
        [np.asarray(out.results[b]["out"]).reshape(()) for b in range(B)],
        dtype=np.float32,
    )
